# revision 21
# baseline (speedup 1.0000x reference)
"""Trainium2 Bass kernel for the co-attention module (nn_Attn_30107720745210).

Contract: kernel(**full_inputs) -> (v_hat, q_hat), both [64, 1024] f32.
Internally: data-parallel over batch across 8 NeuronCores (8 batches/core),
weights replicated. Per batch on-device:
    S  = W @ x2^T                  [1024, 512]
    C  = tanh(x1 @ S)              [512, 512]
    V  = x1 @ Wv ; Q = x2 @ Wq     [512, 512]
    Hv = tanh(V + C @ Q) ; Hq = tanh(Q + C^T @ V)
    sv = Hv @ w_hv ; sq = Hq @ w_hq
    attn = masked-softmax(s, m);  v_hat = attn_v @ x1 ; q_hat = attn_q @ x2
The masked softmax is folded algebraically:
    v_hat = (e*m) @ x1 / (sum(e*m) + 1e-13*sum(e)),  e = exp(s*m)
which matches the reference exactly in real arithmetic.

Matmuls run in float32r (TF32-like, full PE rate). The BIR verifier
requires every f32r matmul operand to be *produced* as f32r, so all
matmul-feeding tiles are typed f32r (DMA'd via bitcast or rounded on
write by the producing ACT/DVE op). PSUM accumulation stays fp32.
"""

import os
from contextlib import ExitStack

import numpy as np

N_CORES = 8
B = 64
BPC = B // N_CORES  # batches per core
L = 512             # L1 == L2
D = 1024            # OUT
A = 512             # ATTN
P = 128             # partitions
LT = L // P         # 4 l-tiles
DT = D // P         # 8 d-tiles

_CACHE = {}


def _build_program(mm_dtype_name: str, split_add: bool):
    import concourse.tile as tile
    from concourse import bacc, mybir
    from concourse.masks import make_identity

    f32 = mybir.dt.float32
    mmdt = getattr(mybir.dt, mm_dtype_name)
    AF = mybir.ActivationFunctionType
    OP = mybir.AluOpType

    def bc(ap):
        return ap.bitcast(mmdt) if mmdt != f32 else ap

    nc = bacc.Bacc("TRN2", target_bir_lowering=False, debug=False)

    x1t = nc.dram_tensor("x1t", [BPC, D, L], f32, kind="ExternalInput").ap()
    x2t = nc.dram_tensor("x2t", [BPC, D, L], f32, kind="ExternalInput").ap()
    x1n = nc.dram_tensor("x1n", [BPC, L, D], f32, kind="ExternalInput").ap()
    x2n = nc.dram_tensor("x2n", [BPC, L, D], f32, kind="ExternalInput").ap()
    m1 = nc.dram_tensor("m1", [BPC, L], f32, kind="ExternalInput").ap()
    m2 = nc.dram_tensor("m2", [BPC, L], f32, kind="ExternalInput").ap()
    wt = nc.dram_tensor("wt", [D, D], f32, kind="ExternalInput").ap()
    wv = nc.dram_tensor("wv", [D, A], f32, kind="ExternalInput").ap()
    wq = nc.dram_tensor("wq", [D, A], f32, kind="ExternalInput").ap()
    whv = nc.dram_tensor("whv", [P, A], f32, kind="ExternalInput").ap()
    whq = nc.dram_tensor("whq", [P, A], f32, kind="ExternalInput").ap()
    vh = nc.dram_tensor("vh", [BPC, D], f32, kind="ExternalOutput").ap()
    qh = nc.dram_tensor("qh", [BPC, D], f32, kind="ExternalOutput").ap()

    with ExitStack() as ctx:
        tc = ctx.enter_context(tile.TileContext(nc))

        const = ctx.enter_context(tc.tile_pool(name="const", bufs=1))
        in1 = ctx.enter_context(tc.tile_pool(name="in1", bufs=2))
        in2 = ctx.enter_context(tc.tile_pool(name="in2", bufs=2))
        sS = ctx.enter_context(tc.tile_pool(name="sS", bufs=1))
        sC = ctx.enter_context(tc.tile_pool(name="sC", bufs=4))
        sCt = ctx.enter_context(tc.tile_pool(name="sCt", bufs=4))
        sV = ctx.enter_context(tc.tile_pool(name="sV", bufs=4))
        sQ = ctx.enter_context(tc.tile_pool(name="sQ", bufs=4))
        sm = ctx.enter_context(tc.tile_pool(name="sm", bufs=8))
        out_pool = ctx.enter_context(tc.tile_pool(name="outp", bufs=2))
        scrp = ctx.enter_context(tc.tile_pool(name="scrp", bufs=2))
        ps = ctx.enter_context(tc.tile_pool(name="ps", bufs=8, space="PSUM"))

        # ---- static loads (matmul operands typed mmdt) ----
        # DMA emission order is the model's (and roughly HW's) service
        # order, so put the startup-critical transfers first: W^T chunk 0,
        # then batch-0 inputs, then everything else. W^T is split into 8
        # per-k-chunk DMAs so the first S matmul only waits for chunk 0.
        WTk = []
        for k in range(DT):
            WTk.append(const.tile([P, D], mmdt, tag=f"WT{k}", name="WTk"))
        X1T_pre = in1.tile([P, DT, L], mmdt, tag="x1", name="X1T_pre")
        X2T_pre = in2.tile([P, DT, L], mmdt, tag="x2", name="X2T_pre")
        # Interleave W^T and x2^T chunk loads in k order: batch 0's S runs
        # k-major, consuming pairs as they land, so the PE starts ~2us in.
        x2t0 = bc(x2t[0].rearrange("(k p) l -> p k l", p=P))
        for k in range(DT):
            nc.sync.dma_start(out=WTk[k], in_=bc(wt[k * P:(k + 1) * P, :]))
            nc.sync.dma_start(out=X2T_pre[:, k, :], in_=x2t0[:, k, :])
        nc.sync.dma_start(out=X1T_pre,
                          in_=bc(x1t[0].rearrange("(k p) l -> p k l", p=P)))
        WV = const.tile([P, DT, A], mmdt, tag="WV")
        nc.sync.dma_start(out=WV, in_=bc(wv.rearrange("(k p) a -> p k a", p=P)))
        WQ = const.tile([P, DT, A], mmdt, tag="WQ")
        nc.sync.dma_start(out=WQ, in_=bc(wq.rearrange("(k p) a -> p k a", p=P)))
        WHV = const.tile([P, A], f32, tag="WHV")
        nc.sync.dma_start(out=WHV, in_=whv)
        WHQ = const.tile([P, A], f32, tag="WHQ")
        nc.sync.dma_start(out=WHQ, in_=whq)
        M1 = const.tile([P, BPC, LT], f32, tag="M1")    # M1[p, b, t] = m1[b, 128t+p]
        nc.sync.dma_start(out=M1, in_=m1.rearrange("b (t p) -> p b t", p=P))
        M2 = const.tile([P, BPC, LT], f32, tag="M2")
        nc.sync.dma_start(out=M2, in_=m2.rearrange("b (t p) -> p b t", p=P))
        IDN = const.tile([P, P], f32, tag="IDN")
        make_identity(nc, IDN)
        if mmdt != f32:
            IDNr = const.tile([P, P], mmdt, tag="IDNr")
            nc.vector.tensor_copy(IDNr, IDN)
        else:
            IDNr = IDN

        n_batches = int(os.environ.get("BK_NBATCH", str(BPC)))
        pipeline = os.environ.get("BK_PIPELINE", "1") == "1"

        def emit_tail(st):
            """Batch tail: normalization + v_hat/q_hat matmuls. Emitted
            mid-way through the NEXT batch's head so the PE never waits on
            the score pipeline."""
            b = st["b"]
            sums = st["sums"]  # [1,4] = [d1_v, sem_v, d1_q, sem_q]
            # denom = sem + 1e-13 * d1 ; rden = 1/denom
            den = sm.tile([1, 4], f32, tag="den")
            nc.vector.tensor_scalar_mul(den[:, 0:1], sums[:, 0:1], 1e-13)
            nc.vector.tensor_add(den[:, 1:2], den[:, 0:1], sums[:, 1:2])
            nc.vector.tensor_scalar_mul(den[:, 2:3], sums[:, 2:3], 1e-13)
            nc.vector.tensor_add(den[:, 3:4], den[:, 2:3], sums[:, 3:4])
            rden = sm.tile([1, 2], f32, tag="rden")
            nc.vector.reciprocal(rden[:, 0:1], den[:, 1:2])
            nc.vector.reciprocal(rden[:, 1:2], den[:, 3:4])

            # v_hat = (em_v @ x1) * rden_v ; q_hat likewise
            vh_sb = out_pool.tile([1, D], f32, tag="vh")
            qh_sb = out_pool.tile([1, D], f32, tag="qh")
            for (em_w, XN, rcol, osb) in (
                (st["emr_v"], st["X1N"], 0, vh_sb),
                (st["emr_q"], st["X2N"], 1, qh_sb),
            ):
                for h in range(2):
                    psh2 = ps.tile([1, 512], f32, tag="ps", name="psh2")
                    for t in range(LT):
                        nc.tensor.matmul(
                            psh2,
                            em_w[:, t:t + 1],
                            XN[:, t, h * 512:(h + 1) * 512],
                            start=(t == 0), stop=(t == LT - 1),
                        )
                    nc.scalar.mul(osb[:, h * 512:(h + 1) * 512], psh2,
                                  rden[:, rcol:rcol + 1])
            nc.sync.dma_start(out=vh[b:b + 1, :], in_=vh_sb)
            nc.sync.dma_start(out=qh[b:b + 1, :], in_=qh_sb)

        repeat = int(os.environ.get("BK_REPEAT", "1"))
        pending_tail = None
        for b_outer in range(n_batches * repeat):
            b = b_outer % n_batches
            if b_outer == 0:
                X1T, X2T = X1T_pre, X2T_pre
            else:
                X1T, X2T = prefetched

            # ---- S = W @ x2^T : [d, m], 8 tiles [128, 512] ----
            S = sS.tile([P, DT, L], mmdt, tag="S")
            if b_outer == 0:
                # k-major so matmuls consume (WTk, x2T chunk) pairs in DMA
                # arrival order; needs all 8 psum banks, which are free at
                # kernel start.
                psjs = [ps.tile([P, L], f32, tag="ps", name="psj0")
                        for _ in range(DT)]
                for k in range(DT):
                    for j in range(DT):
                        nc.tensor.matmul(
                            psjs[j],
                            WTk[k][:, j * P:(j + 1) * P],
                            X2T[:, k, :],
                            start=(k == 0), stop=(k == DT - 1),
                        )
                for j in range(DT):
                    nc.vector.tensor_copy(S[:, j, :], psjs[j])
            else:
                for j in range(DT):
                    psj = ps.tile([P, L], f32, tag="ps")
                    for k in range(DT):
                        nc.tensor.matmul(
                            psj,
                            WTk[k][:, j * P:(j + 1) * P],
                            X2T[:, k, :],
                            start=(k == 0), stop=(k == DT - 1),
                        )
                    nc.vector.tensor_copy(S[:, j, :], psj)

            # previous batch's tail goes here: its operands are ready by now
            # and the S matmuls above gave DVE/ACT time to finish the scores.
            if pending_tail is not None and pipeline:
                emit_tail(pending_tail)
                pending_tail = None

            # ---- C = tanh(x1 @ S) : [l, m], 4 tiles [128, 512] ----
            C = [None] * LT
            for i in range(LT):
                psc = ps.tile([P, L], f32, tag="ps")
                for k in range(DT):
                    nc.tensor.matmul(
                        psc,
                        X1T[:, k, i * P:(i + 1) * P],
                        S[:, k, :],
                        start=(k == 0), stop=(k == DT - 1),
                    )
                C[i] = sC.tile([P, L], mmdt, tag="C", name="C_l")
                nc.scalar.activation(C[i], psc, AF.Tanh)

            # ---- V = x1 @ Wv (psum kept for Hv accumulation) ----
            psV = [None] * LT
            V = [None] * LT
            for i in range(LT):
                psV[i] = ps.tile([P, A], f32, tag="ps", name="psV")
                for k in range(DT):
                    nc.tensor.matmul(
                        psV[i],
                        X1T[:, k, i * P:(i + 1) * P],
                        WV[:, k, :],
                        start=(k == 0), stop=(split_add and k == DT - 1),
                        skip_group_check=not split_add,
                    )
                V[i] = sV.tile([P, A], mmdt, tag="V", name="V")
                nc.scalar.copy(V[i], psV[i])

            # ---- C^T via PE transpose: Ct[j][:, i*128:] = C[i][:, j*128:].T ----
            Ct = [None] * LT
            for j in range(LT):
                pst = ps.tile([P, L], mmdt, tag="ps")
                for i in range(LT):
                    nc.tensor.transpose(
                        pst[:, i * P:(i + 1) * P], C[i][:, j * P:(j + 1) * P], IDNr
                    )
                Ct[j] = sCt.tile([P, L], mmdt, tag="Ct", name="Ct")
                nc.vector.tensor_copy(Ct[j], pst)

            # ---- Q = x2 @ Wq (psum kept) ----
            psQ = [None] * LT
            Q = [None] * LT
            for i in range(LT):
                psQ[i] = ps.tile([P, A], f32, tag="ps", name="psQ")
                for k in range(DT):
                    nc.tensor.matmul(
                        psQ[i],
                        X2T[:, k, i * P:(i + 1) * P],
                        WQ[:, k, :],
                        start=(k == 0), stop=(split_add and k == DT - 1),
                        skip_group_check=not split_add,
                    )
                Q[i] = sQ.tile([P, A], mmdt, tag="Q", name="Q")
                nc.scalar.copy(Q[i], psQ[i])

            # prefetch next batch's transposed inputs ahead of this
            # batch's natural-layout loads (DMAs drain in emission order;
            # X1T/X2T gate the next batch's S matmuls, X1N/X2N only gate
            # the deferred tail)
            if b_outer + 1 < n_batches * repeat:
                nb_ = (b_outer + 1) % n_batches
                X1T_nxt = in1.tile([P, DT, L], mmdt, tag="x1", name="X1T_nxt")
                nc.sync.dma_start(
                    out=X1T_nxt,
                    in_=bc(x1t[nb_].rearrange("(k p) l -> p k l", p=P)))
                X2T_nxt = in2.tile([P, DT, L], mmdt, tag="x2", name="X2T_nxt")
                nc.sync.dma_start(
                    out=X2T_nxt,
                    in_=bc(x2t[nb_].rearrange("(k p) l -> p k l", p=P)))
                prefetched = (X1T_nxt, X2T_nxt)
            else:
                prefetched = None

            # x1 natural layout, for v_hat (reuses the x1 pool slots)
            X1N = in1.tile([P, LT, D], mmdt, tag="x1")
            nc.sync.dma_start(out=X1N,
                              in_=bc(x1n[b].rearrange("(t p) d -> p t d", p=P)))
            X2N = in2.tile([P, LT, D], mmdt, tag="x2")
            nc.sync.dma_start(out=X2N,
                              in_=bc(x2n[b].rearrange("(t p) d -> p t d", p=P)))

            # ---- Hv = tanh(V + C@Q) accumulated onto psV; Hq likewise.
            # tanh writes SBUF (pool-shared with Ct/C slots) so the psum
            # banks free as soon as ACT reads them. ----
            def h_block(psH, lhs_tiles, rhs_tiles, hpool, htag):
                H = [None] * LT
                for i in range(LT):
                    if split_add:
                        psh = ps.tile([P, A], f32, tag="ps", name="psh")
                    else:
                        psh = psH[i]
                    for j in range(LT):
                        nc.tensor.matmul(
                            psh,
                            lhs_tiles[j][:, i * P:(i + 1) * P],
                            rhs_tiles[j],
                            start=(split_add and j == 0),
                            stop=(j == LT - 1),
                            skip_group_check=not split_add,
                        )
                    if split_add:
                        nc.vector.tensor_add(psH[i], psH[i], psh)
                    H[i] = hpool.tile([P, A], f32, tag=htag, name="H")
                    nc.scalar.activation(H[i], psH[i], AF.Tanh)
                return H

            Hv = h_block(psV, Ct, Q, sCt, "Ct")
            Hq = h_block(psQ, C, V, sC, "C")

            # ---- scores: sv[l] = Hv[l,:] . w_hv ----
            # (DVE multiply, then ACT Copy-with-accumulate reduce;
            #  InstTensorTensorReduce crashes the exec unit on this HW)
            sv = sm.tile([P, LT], f32, tag="sv")
            sq = sm.tile([P, LT], f32, tag="sq")
            for i in range(LT):
                scr_v = scrp.tile([P, A], f32, tag="scr", name="scr_v")
                nc.vector.tensor_mul(scr_v, Hv[i], WHV)
                nc.scalar.activation(scr_v, scr_v, AF.Copy,
                                     accum_out=sv[:, i:i + 1])
                scr_q = scrp.tile([P, A], f32, tag="scr", name="scr_q")
                nc.vector.tensor_mul(scr_q, Hq[i], WHQ)
                nc.scalar.activation(scr_q, scr_q, AF.Copy,
                                     accum_out=sq[:, i:i + 1])

            # ---- masked softmax weights ----
            # e/em for both sides live in one [128, 4, LT] tile so a single
            # GPSIMD partition-reduce + DVE free-reduce replaces 16 PE
            # matmuls for the column sums.
            eall = sm.tile([P, 4, LT], f32, tag="eall", name="eall")

            def attn_weights(s, M, slot, tag):
                smv = sm.tile([P, LT], f32, tag=tag + "_sm", name="smv")
                nc.vector.tensor_mul(smv, s, M[:, b, :])
                nc.scalar.activation(eall[:, slot, :], smv, AF.Exp)
                nc.vector.tensor_mul(eall[:, slot + 1, :], eall[:, slot, :],
                                     M[:, b, :])
                em_r = sm.tile([P, LT], mmdt, tag=tag + "_emr", name="em_r")
                nc.vector.tensor_copy(em_r, eall[:, slot + 1, :])
                return em_r

            emr_v = attn_weights(sv, M1, 0, "v")
            emr_q = attn_weights(sq, M2, 2, "q")
            # partition sums: [128, 4*LT] -> [1, 4*LT] -> [1, 4]
            psum16 = sm.tile([1, 4, LT], f32, tag="psum16", name="psum16")
            nc.gpsimd.tensor_reduce(out=psum16.rearrange("o f l -> o (f l)"),
                                    in_=eall.rearrange("p f l -> p (f l)"),
                                    axis=mybir.AxisListType.C, op=OP.add)
            sums = sm.tile([1, 4], f32, tag="sums", name="sums")
            nc.vector.tensor_reduce(out=sums, in_=psum16,
                                    axis=mybir.AxisListType.X, op=OP.add)

            st = {"b": b, "sums": sums,
                  "emr_v": emr_v, "emr_q": emr_q, "X1N": X1N, "X2N": X2N}
            if pipeline:
                pending_tail = st
            else:
                emit_tail(st)
        if pending_tail is not None:
            emit_tail(pending_tail)

    nc.compile()
    return nc


def _get_program():
    mm_dtype = os.environ.get("BK_MM_DTYPE", "float32r")
    split_add = os.environ.get("BK_SPLIT_ADD", "0") == "1"
    key = (mm_dtype, split_add)
    if key not in _CACHE:
        _CACHE[key] = _build_program(mm_dtype, split_add)
    return _CACHE[key]


def _make_in_maps(seq_features1, seq_features2, mask1, mask2, W, Wv, Wq, w_hv, w_hq):
    x1 = np.ascontiguousarray(seq_features1, dtype=np.float32)
    x2 = np.ascontiguousarray(seq_features2, dtype=np.float32)
    x1t = np.ascontiguousarray(x1.transpose(0, 2, 1))
    x2t = np.ascontiguousarray(x2.transpose(0, 2, 1))
    m1 = np.ascontiguousarray(mask1, dtype=np.float32)
    m2 = np.ascontiguousarray(mask2, dtype=np.float32)
    wt = np.ascontiguousarray(np.asarray(W, dtype=np.float32).T)
    wv = np.ascontiguousarray(Wv, dtype=np.float32)
    wq = np.ascontiguousarray(Wq, dtype=np.float32)
    whv = np.ascontiguousarray(
        np.broadcast_to(np.asarray(w_hv, np.float32).reshape(1, A), (P, A)))
    whq = np.ascontiguousarray(
        np.broadcast_to(np.asarray(w_hq, np.float32).reshape(1, A), (P, A)))

    in_maps = []
    for c in range(N_CORES):
        sl = slice(c * BPC, (c + 1) * BPC)
        in_maps.append({
            "x1t": x1t[sl], "x2t": x2t[sl],
            "x1n": x1[sl], "x2n": x2[sl],
            "m1": m1[sl], "m2": m2[sl],
            "wt": wt, "wv": wv, "wq": wq, "whv": whv, "whq": whq,
        })
    return in_maps


def run_on_hw(in_maps, trace=False, **kw):
    from concourse import bass_utils
    nc = _get_program()
    return bass_utils.run_bass_kernel_spmd(
        nc, in_maps, core_ids=list(range(N_CORES)), trace=trace, **kw)


def _gather(res):
    v_hat = np.concatenate([res.results[c]["vh"] for c in range(N_CORES)], axis=0)
    q_hat = np.concatenate([res.results[c]["qh"] for c in range(N_CORES)], axis=0)
    return (v_hat, q_hat)


def _run_inproc(in_maps):
    return _gather(run_on_hw(in_maps))


def _child_main(in_path, out_path):
    z = np.load(in_path)
    n = int(z["n_maps"])
    in_maps = []
    for c in range(n):
        in_maps.append({k[len(f"c{c}_"):]: z[k] for k in z.files
                        if k.startswith(f"c{c}_")})
    v_hat, q_hat = _run_inproc(in_maps)
    np.savez(out_path, v_hat=v_hat, q_hat=q_hat)


def _run_subprocess(in_maps):
    import subprocess
    import sys
    import tempfile
    d = tempfile.mkdtemp(prefix="bk_sub_")
    in_path = os.path.join(d, "in.npz")
    out_path = os.path.join(d, "out.npz")
    payload = {"n_maps": np.int64(len(in_maps))}
    for c, m in enumerate(in_maps):
        for k, v in m.items():
            payload[f"c{c}_{k}"] = v
    np.savez(in_path, **payload)
    here = os.path.dirname(os.path.abspath(__file__))
    code = (f"import sys; sys.path.insert(0, {here!r}); "
            f"import kernel; kernel._child_main({in_path!r}, {out_path!r})")
    subprocess.run([sys.executable, "-c", code], check=True, timeout=1800)
    z = np.load(out_path)
    return (z["v_hat"], z["q_hat"])


def kernel(seq_features1, seq_features2, mask1, mask2, W, Wv, Wq, w_hv, w_hq):
    in_maps = _make_in_maps(
        seq_features1, seq_features2, mask1, mask2, W, Wv, Wq, w_hv, w_hq)
    # The axon-attached NeuronCores are occasionally left in an
    # NRT_EXEC_UNIT_UNRECOVERABLE state by a prior process; a fresh process
    # (not an in-process retry) heals it. Try fast in-process first, then
    # fall back to fresh-subprocess attempts.
    import time
    try:
        return _run_inproc(in_maps)
    except Exception:
        pass
    last = None
    for _ in range(3):
        time.sleep(30)
        try:
            return _run_subprocess(in_maps)
        except Exception as e:  # noqa: BLE001 - retry any device failure
            last = e
    raise last


# revision 24
# speedup vs baseline: 1.0142x; 1.0142x over previous
"""Trainium2 Bass kernel for the co-attention module (nn_Attn_30107720745210).

Contract: kernel(**full_inputs) -> (v_hat, q_hat), both [64, 1024] f32.
Internally: data-parallel over batch across 8 NeuronCores (8 batches/core),
weights replicated. Per batch on-device:
    S  = W @ x2^T                  [1024, 512]
    C  = tanh(x1 @ S)              [512, 512]
    V  = x1 @ Wv ; Q = x2 @ Wq     [512, 512]
    Hv = tanh(V + C @ Q) ; Hq = tanh(Q + C^T @ V)
    sv = Hv @ w_hv ; sq = Hq @ w_hq
    attn = masked-softmax(s, m);  v_hat = attn_v @ x1 ; q_hat = attn_q @ x2
The masked softmax is folded algebraically:
    v_hat = (e*m) @ x1 / (sum(e*m) + 1e-13*sum(e)),  e = exp(s*m)
which matches the reference exactly in real arithmetic.

Matmuls run in float32r (TF32-like, full PE rate). The BIR verifier
requires every f32r matmul operand to be *produced* as f32r, so all
matmul-feeding tiles are typed f32r (DMA'd via bitcast or rounded on
write by the producing ACT/DVE op). PSUM accumulation stays fp32.
"""

import os
from contextlib import ExitStack

import numpy as np

N_CORES = 8
B = 64
BPC = B // N_CORES  # batches per core
L = 512             # L1 == L2
D = 1024            # OUT
A = 512             # ATTN
P = 128             # partitions
LT = L // P         # 4 l-tiles
DT = D // P         # 8 d-tiles

_CACHE = {}


def _build_program(mm_dtype_name: str, split_add: bool):
    import concourse.tile as tile
    from concourse import bacc, mybir
    from concourse.masks import make_identity

    f32 = mybir.dt.float32
    mmdt = getattr(mybir.dt, mm_dtype_name)
    AF = mybir.ActivationFunctionType
    OP = mybir.AluOpType

    def bc(ap):
        return ap.bitcast(mmdt) if mmdt != f32 else ap

    nc = bacc.Bacc("TRN2", target_bir_lowering=False, debug=False)

    x1t = nc.dram_tensor("x1t", [BPC, D, L], f32, kind="ExternalInput").ap()
    x2t = nc.dram_tensor("x2t", [BPC, D, L], f32, kind="ExternalInput").ap()
    x1n = nc.dram_tensor("x1n", [BPC, L, D], f32, kind="ExternalInput").ap()
    x2n = nc.dram_tensor("x2n", [BPC, L, D], f32, kind="ExternalInput").ap()
    m1 = nc.dram_tensor("m1", [BPC, L], f32, kind="ExternalInput").ap()
    m2 = nc.dram_tensor("m2", [BPC, L], f32, kind="ExternalInput").ap()
    wt = nc.dram_tensor("wt", [D, D], f32, kind="ExternalInput").ap()
    wv = nc.dram_tensor("wv", [D, A], f32, kind="ExternalInput").ap()
    wq = nc.dram_tensor("wq", [D, A], f32, kind="ExternalInput").ap()
    whv = nc.dram_tensor("whv", [P, A], f32, kind="ExternalInput").ap()
    whq = nc.dram_tensor("whq", [P, A], f32, kind="ExternalInput").ap()
    vh = nc.dram_tensor("vh", [BPC, D], f32, kind="ExternalOutput").ap()
    qh = nc.dram_tensor("qh", [BPC, D], f32, kind="ExternalOutput").ap()

    with ExitStack() as ctx:
        tc = ctx.enter_context(tile.TileContext(nc))

        const = ctx.enter_context(tc.tile_pool(name="const", bufs=1))
        in1 = ctx.enter_context(tc.tile_pool(name="in1", bufs=2))
        in2 = ctx.enter_context(tc.tile_pool(name="in2", bufs=2))
        sS = ctx.enter_context(tc.tile_pool(name="sS", bufs=1))
        sC = ctx.enter_context(tc.tile_pool(name="sC", bufs=4))
        sCt = ctx.enter_context(tc.tile_pool(name="sCt", bufs=4))
        sV = ctx.enter_context(tc.tile_pool(name="sV", bufs=4))
        sQ = ctx.enter_context(tc.tile_pool(name="sQ", bufs=4))
        sm = ctx.enter_context(tc.tile_pool(name="sm", bufs=8))
        out_pool = ctx.enter_context(tc.tile_pool(name="outp", bufs=2))
        scrp = ctx.enter_context(tc.tile_pool(name="scrp", bufs=2))
        ps = ctx.enter_context(tc.tile_pool(name="ps", bufs=8, space="PSUM"))

        # ---- static loads (matmul operands typed mmdt) ----
        # DMA emission order is the model's (and roughly HW's) service
        # order, so put the startup-critical transfers first: W^T chunk 0,
        # then batch-0 inputs, then everything else. W^T is split into 8
        # per-k-chunk DMAs so the first S matmul only waits for chunk 0.
        WTk = []
        for k in range(DT):
            WTk.append(const.tile([P, D], mmdt, tag=f"WT{k}", name="WTk"))
        X1T_pre = in1.tile([P, DT, L], mmdt, tag="x1", name="X1T_pre")
        X2T_pre = in2.tile([P, DT, L], mmdt, tag="x2", name="X2T_pre")
        # Interleave W^T and x2^T chunk loads in k order: batch 0's S runs
        # k-major, consuming pairs as they land, so the PE starts ~2us in.
        x2t0 = bc(x2t[0].rearrange("(k p) l -> p k l", p=P))
        for k in range(DT):
            nc.sync.dma_start(out=WTk[k], in_=bc(wt[k * P:(k + 1) * P, :]))
            nc.sync.dma_start(out=X2T_pre[:, k, :], in_=x2t0[:, k, :])
        nc.sync.dma_start(out=X1T_pre,
                          in_=bc(x1t[0].rearrange("(k p) l -> p k l", p=P)))
        WV = const.tile([P, DT, A], mmdt, tag="WV")
        nc.sync.dma_start(out=WV, in_=bc(wv.rearrange("(k p) a -> p k a", p=P)))
        WQ = const.tile([P, DT, A], mmdt, tag="WQ")
        nc.sync.dma_start(out=WQ, in_=bc(wq.rearrange("(k p) a -> p k a", p=P)))
        WHV = const.tile([P, A], f32, tag="WHV")
        nc.sync.dma_start(out=WHV, in_=whv)
        WHQ = const.tile([P, A], f32, tag="WHQ")
        nc.sync.dma_start(out=WHQ, in_=whq)
        M1 = const.tile([P, BPC, LT], f32, tag="M1")    # M1[p, b, t] = m1[b, 128t+p]
        nc.sync.dma_start(out=M1, in_=m1.rearrange("b (t p) -> p b t", p=P))
        M2 = const.tile([P, BPC, LT], f32, tag="M2")
        nc.sync.dma_start(out=M2, in_=m2.rearrange("b (t p) -> p b t", p=P))
        IDN = const.tile([P, P], f32, tag="IDN")
        make_identity(nc, IDN)
        if mmdt != f32:
            IDNr = const.tile([P, P], mmdt, tag="IDNr")
            nc.vector.tensor_copy(IDNr, IDN)
        else:
            IDNr = IDN

        n_batches = int(os.environ.get("BK_NBATCH", str(BPC)))
        pipeline = os.environ.get("BK_PIPELINE", "1") == "1"

        def emit_tail(st):
            """Batch tail: normalization + v_hat/q_hat matmuls. Emitted
            mid-way through the NEXT batch's head so the PE never waits on
            the score pipeline."""
            b = st["b"]
            sums = st["sums"]  # [1,4] = [d1_v, sem_v, d1_q, sem_q]
            # denom = sem + 1e-13 * d1 ; rden = 1/denom
            den = sm.tile([1, 4], f32, tag="den")
            nc.vector.tensor_scalar_mul(den[:, 0:1], sums[:, 0:1], 1e-13)
            nc.vector.tensor_add(den[:, 1:2], den[:, 0:1], sums[:, 1:2])
            nc.vector.tensor_scalar_mul(den[:, 2:3], sums[:, 2:3], 1e-13)
            nc.vector.tensor_add(den[:, 3:4], den[:, 2:3], sums[:, 3:4])
            rden = sm.tile([1, 2], f32, tag="rden")
            nc.vector.reciprocal(rden[:, 0:1], den[:, 1:2])
            nc.vector.reciprocal(rden[:, 1:2], den[:, 3:4])

            # v_hat = (em_v @ x1) * rden_v ; q_hat likewise
            vh_sb = out_pool.tile([1, D], f32, tag="vh")
            qh_sb = out_pool.tile([1, D], f32, tag="qh")
            for (em_w, XN, rcol, osb) in (
                (st["emr_v"], st["X1N"], 0, vh_sb),
                (st["emr_q"], st["X2N"], 1, qh_sb),
            ):
                for h in range(2):
                    psh2 = ps.tile([1, 512], f32, tag="ps", name="psh2")
                    for t in range(LT):
                        nc.tensor.matmul(
                            psh2,
                            em_w[:, t:t + 1],
                            XN[:, t, h * 512:(h + 1) * 512],
                            start=(t == 0), stop=(t == LT - 1),
                        )
                    nc.scalar.mul(osb[:, h * 512:(h + 1) * 512], psh2,
                                  rden[:, rcol:rcol + 1])
            nc.sync.dma_start(out=vh[b:b + 1, :], in_=vh_sb)
            nc.sync.dma_start(out=qh[b:b + 1, :], in_=qh_sb)

        repeat = int(os.environ.get("BK_REPEAT", "1"))
        pending_tail = None
        for b_outer in range(n_batches * repeat):
            b = b_outer % n_batches
            if b_outer == 0:
                X1T, X2T = X1T_pre, X2T_pre
            else:
                X1T, X2T = prefetched

            # ---- S = W @ x2^T : [d, m], 8 tiles [128, 512] ----
            S = sS.tile([P, DT, L], mmdt, tag="S")
            if b_outer == 0:
                # k-major so matmuls consume (WTk, x2T chunk) pairs in DMA
                # arrival order; needs all 8 psum banks, which are free at
                # kernel start.
                psjs = [ps.tile([P, L], f32, tag="ps", name="psj0")
                        for _ in range(DT)]
                for k in range(DT):
                    for j in range(DT):
                        nc.tensor.matmul(
                            psjs[j],
                            WTk[k][:, j * P:(j + 1) * P],
                            X2T[:, k, :],
                            start=(k == 0), stop=(k == DT - 1),
                        )
                for j in range(DT):
                    nc.vector.tensor_copy(S[:, j, :], psjs[j])
            else:
                for j in range(DT):
                    psj = ps.tile([P, L], f32, tag="ps")
                    for k in range(DT):
                        nc.tensor.matmul(
                            psj,
                            WTk[k][:, j * P:(j + 1) * P],
                            X2T[:, k, :],
                            start=(k == 0), stop=(k == DT - 1),
                        )
                    nc.vector.tensor_copy(S[:, j, :], psj)

            # previous batch's tail goes here: its operands are ready by now
            # and the S matmuls above gave DVE/ACT time to finish the scores.
            if pending_tail is not None and pipeline:
                emit_tail(pending_tail)
                pending_tail = None

            # ---- C = tanh(x1 @ S) : [l, m], 4 tiles [128, 512] ----
            C = [None] * LT
            for i in range(LT):
                psc = ps.tile([P, L], f32, tag="ps")
                for k in range(DT):
                    nc.tensor.matmul(
                        psc,
                        X1T[:, k, i * P:(i + 1) * P],
                        S[:, k, :],
                        start=(k == 0), stop=(k == DT - 1),
                    )
                C[i] = sC.tile([P, L], mmdt, tag="C", name="C_l")
                nc.scalar.activation(C[i], psc, AF.Tanh)

            # ---- V = x1 @ Wv (psum kept for Hv accumulation) ----
            psV = [None] * LT
            V = [None] * LT
            for i in range(LT):
                psV[i] = ps.tile([P, A], f32, tag="ps", name="psV")
                for k in range(DT):
                    nc.tensor.matmul(
                        psV[i],
                        X1T[:, k, i * P:(i + 1) * P],
                        WV[:, k, :],
                        start=(k == 0), stop=(split_add and k == DT - 1),
                        skip_group_check=not split_add,
                    )
                V[i] = sV.tile([P, A], mmdt, tag="V", name="V")
                nc.scalar.copy(V[i], psV[i])

            # ---- C^T via PE transpose: Ct[j][:, i*128:] = C[i][:, j*128:].T ----
            Ct = [None] * LT
            for j in range(LT):
                pst = ps.tile([P, L], mmdt, tag="ps")
                for i in range(LT):
                    nc.tensor.transpose(
                        pst[:, i * P:(i + 1) * P], C[i][:, j * P:(j + 1) * P], IDNr
                    )
                Ct[j] = sCt.tile([P, L], mmdt, tag="Ct", name="Ct")
                nc.vector.tensor_copy(Ct[j], pst)

            # ---- Q = x2 @ Wq (psum kept) ----
            psQ = [None] * LT
            Q = [None] * LT
            for i in range(LT):
                psQ[i] = ps.tile([P, A], f32, tag="ps", name="psQ")
                for k in range(DT):
                    nc.tensor.matmul(
                        psQ[i],
                        X2T[:, k, i * P:(i + 1) * P],
                        WQ[:, k, :],
                        start=(k == 0), stop=(split_add and k == DT - 1),
                        skip_group_check=not split_add,
                    )
                Q[i] = sQ.tile([P, A], mmdt, tag="Q", name="Q")
                nc.scalar.copy(Q[i], psQ[i])

            # prefetch next batch's transposed inputs ahead of this
            # batch's natural-layout loads (DMAs drain in emission order;
            # X1T/X2T gate the next batch's S matmuls, X1N/X2N only gate
            # the deferred tail)
            if b_outer + 1 < n_batches * repeat:
                nb_ = (b_outer + 1) % n_batches
                X1T_nxt = in1.tile([P, DT, L], mmdt, tag="x1", name="X1T_nxt")
                nc.sync.dma_start(
                    out=X1T_nxt,
                    in_=bc(x1t[nb_].rearrange("(k p) l -> p k l", p=P)))
                X2T_nxt = in2.tile([P, DT, L], mmdt, tag="x2", name="X2T_nxt")
                nc.sync.dma_start(
                    out=X2T_nxt,
                    in_=bc(x2t[nb_].rearrange("(k p) l -> p k l", p=P)))
                prefetched = (X1T_nxt, X2T_nxt)
            else:
                prefetched = None

            # x1 natural layout, for v_hat (reuses the x1 pool slots)
            X1N = in1.tile([P, LT, D], mmdt, tag="x1")
            nc.sync.dma_start(out=X1N,
                              in_=bc(x1n[b].rearrange("(t p) d -> p t d", p=P)))
            X2N = in2.tile([P, LT, D], mmdt, tag="x2")
            nc.sync.dma_start(out=X2N,
                              in_=bc(x2n[b].rearrange("(t p) d -> p t d", p=P)))

            # ---- Hv = tanh(V + C@Q) accumulated onto psV; Hq likewise.
            # tanh writes SBUF (pool-shared with Ct/C slots) so the psum
            # banks free as soon as ACT reads them. ----
            # Score reduction (DVE multiply + ACT Copy-with-accumulate;
            # InstTensorTensorReduce crashes the exec unit on this HW) is
            # interleaved per l-tile inside the H blocks so the softmax
            # chain overlaps the remaining H matmuls instead of running
            # after them — this is what keeps the last batch's tail short.
            def h_block(psH, lhs_tiles, rhs_tiles, hpool, htag, wb, s_out):
                for i in range(LT):
                    if split_add:
                        psh = ps.tile([P, A], f32, tag="ps", name="psh")
                    else:
                        psh = psH[i]
                    for j in range(LT):
                        nc.tensor.matmul(
                            psh,
                            lhs_tiles[j][:, i * P:(i + 1) * P],
                            rhs_tiles[j],
                            start=(split_add and j == 0),
                            stop=(j == LT - 1),
                            skip_group_check=not split_add,
                        )
                    if split_add:
                        nc.vector.tensor_add(psH[i], psH[i], psh)
                    H = hpool.tile([P, A], f32, tag=htag, name="H")
                    nc.scalar.activation(H, psH[i], AF.Tanh)
                    scr = scrp.tile([P, A], f32, tag="scr", name="scr")
                    nc.vector.tensor_mul(scr, H, wb)
                    nc.scalar.activation(scr, scr, AF.Copy,
                                         accum_out=s_out[:, i:i + 1])

            # e/em for both sides live in one [128, 4, LT] tile so a single
            # GPSIMD partition-reduce + DVE free-reduce replaces 16 PE
            # matmuls for the column sums.
            eall = sm.tile([P, 4, LT], f32, tag="eall", name="eall")

            def attn_weights(s, M, slot, tag):
                smv = sm.tile([P, LT], f32, tag=tag + "_sm", name="smv")
                nc.vector.tensor_mul(smv, s, M[:, b, :])
                nc.scalar.activation(eall[:, slot, :], smv, AF.Exp)
                nc.vector.tensor_mul(eall[:, slot + 1, :], eall[:, slot, :],
                                     M[:, b, :])
                em_r = sm.tile([P, LT], mmdt, tag=tag + "_emr", name="em_r")
                nc.vector.tensor_copy(em_r, eall[:, slot + 1, :])
                return em_r

            sv = sm.tile([P, LT], f32, tag="sv")
            sq = sm.tile([P, LT], f32, tag="sq")
            h_block(psV, Ct, Q, sCt, "Ct", WHV, sv)   # Hv + v-scores
            emr_v = attn_weights(sv, M1, 0, "v")
            h_block(psQ, C, V, sC, "C", WHQ, sq)      # Hq + q-scores
            emr_q = attn_weights(sq, M2, 2, "q")
            # partition sums: [128, 4*LT] -> [1, 4*LT] -> [1, 4]
            psum16 = sm.tile([1, 4, LT], f32, tag="psum16", name="psum16")
            nc.gpsimd.tensor_reduce(out=psum16.rearrange("o f l -> o (f l)"),
                                    in_=eall.rearrange("p f l -> p (f l)"),
                                    axis=mybir.AxisListType.C, op=OP.add)
            sums = sm.tile([1, 4], f32, tag="sums", name="sums")
            nc.vector.tensor_reduce(out=sums, in_=psum16,
                                    axis=mybir.AxisListType.X, op=OP.add)

            st = {"b": b, "sums": sums,
                  "emr_v": emr_v, "emr_q": emr_q, "X1N": X1N, "X2N": X2N}
            if pipeline:
                pending_tail = st
            else:
                emit_tail(st)
        if pending_tail is not None:
            emit_tail(pending_tail)

    nc.compile()
    return nc


def _get_program():
    mm_dtype = os.environ.get("BK_MM_DTYPE", "float32r")
    split_add = os.environ.get("BK_SPLIT_ADD", "0") == "1"
    key = (mm_dtype, split_add)
    if key not in _CACHE:
        _CACHE[key] = _build_program(mm_dtype, split_add)
    return _CACHE[key]


def _make_in_maps(seq_features1, seq_features2, mask1, mask2, W, Wv, Wq, w_hv, w_hq):
    x1 = np.ascontiguousarray(seq_features1, dtype=np.float32)
    x2 = np.ascontiguousarray(seq_features2, dtype=np.float32)
    x1t = np.ascontiguousarray(x1.transpose(0, 2, 1))
    x2t = np.ascontiguousarray(x2.transpose(0, 2, 1))
    m1 = np.ascontiguousarray(mask1, dtype=np.float32)
    m2 = np.ascontiguousarray(mask2, dtype=np.float32)
    wt = np.ascontiguousarray(np.asarray(W, dtype=np.float32).T)
    wv = np.ascontiguousarray(Wv, dtype=np.float32)
    wq = np.ascontiguousarray(Wq, dtype=np.float32)
    whv = np.ascontiguousarray(
        np.broadcast_to(np.asarray(w_hv, np.float32).reshape(1, A), (P, A)))
    whq = np.ascontiguousarray(
        np.broadcast_to(np.asarray(w_hq, np.float32).reshape(1, A), (P, A)))

    in_maps = []
    for c in range(N_CORES):
        sl = slice(c * BPC, (c + 1) * BPC)
        in_maps.append({
            "x1t": x1t[sl], "x2t": x2t[sl],
            "x1n": x1[sl], "x2n": x2[sl],
            "m1": m1[sl], "m2": m2[sl],
            "wt": wt, "wv": wv, "wq": wq, "whv": whv, "whq": whq,
        })
    return in_maps


def run_on_hw(in_maps, trace=False, **kw):
    from concourse import bass_utils
    nc = _get_program()
    return bass_utils.run_bass_kernel_spmd(
        nc, in_maps, core_ids=list(range(N_CORES)), trace=trace, **kw)


def _gather(res):
    v_hat = np.concatenate([res.results[c]["vh"] for c in range(N_CORES)], axis=0)
    q_hat = np.concatenate([res.results[c]["qh"] for c in range(N_CORES)], axis=0)
    return (v_hat, q_hat)


def _run_inproc(in_maps):
    return _gather(run_on_hw(in_maps))


def _child_main(in_path, out_path):
    z = np.load(in_path)
    n = int(z["n_maps"])
    in_maps = []
    for c in range(n):
        in_maps.append({k[len(f"c{c}_"):]: z[k] for k in z.files
                        if k.startswith(f"c{c}_")})
    v_hat, q_hat = _run_inproc(in_maps)
    np.savez(out_path, v_hat=v_hat, q_hat=q_hat)


def _run_subprocess(in_maps):
    import subprocess
    import sys
    import tempfile
    d = tempfile.mkdtemp(prefix="bk_sub_")
    in_path = os.path.join(d, "in.npz")
    out_path = os.path.join(d, "out.npz")
    payload = {"n_maps": np.int64(len(in_maps))}
    for c, m in enumerate(in_maps):
        for k, v in m.items():
            payload[f"c{c}_{k}"] = v
    np.savez(in_path, **payload)
    here = os.path.dirname(os.path.abspath(__file__))
    code = (f"import sys; sys.path.insert(0, {here!r}); "
            f"import kernel; kernel._child_main({in_path!r}, {out_path!r})")
    subprocess.run([sys.executable, "-c", code], check=True, timeout=1800)
    z = np.load(out_path)
    return (z["v_hat"], z["q_hat"])


def kernel(seq_features1, seq_features2, mask1, mask2, W, Wv, Wq, w_hv, w_hq):
    in_maps = _make_in_maps(
        seq_features1, seq_features2, mask1, mask2, W, Wv, Wq, w_hv, w_hq)
    # The axon-attached NeuronCores are occasionally left in an
    # NRT_EXEC_UNIT_UNRECOVERABLE state by a prior process; a fresh process
    # (not an in-process retry) heals it. Try fast in-process first, then
    # fall back to fresh-subprocess attempts.
    import time
    try:
        return _run_inproc(in_maps)
    except Exception:
        pass
    last = None
    for _ in range(3):
        time.sleep(30)
        try:
            return _run_subprocess(in_maps)
        except Exception as e:  # noqa: BLE001 - retry any device failure
            last = e
    raise last
